# revision 7
# baseline (speedup 1.0000x reference)
"""Multi-head self-attention on 8 TRN2 NeuronCores.

Sharding: (batch=4) x (query-half=2) = 8 shards, zero collectives.
Each core computes, for its (b, half):
  - K, V for the full sequence of batch b (duplicated across the 2 query-half
    cores of that batch; cheaper than an all-gather at this size)
  - Q for its 1024-query half
  - scoresT = KT_h^T Q per head (no max-subtraction; scores are small by
    construction), exp on ScalarE, PV with an appended ones-column to get the
    softmax denominator for free, normalize, then the output projection.
All matmuls run in fp16 with fp32 PSUM accumulation (validated 3.3e-4 rel err
against the fp32 reference). Layouts are pre-transposed/tiled on the host so
the device never transposes anything.
"""

import os
import numpy as np

B, S, D = 4, 2048, 1024
H, DK = 16, 64
SQ = S // 2          # queries per core
FV = H * 65          # V' columns: per head 64 value dims + 1 ones column
SCALE = 64 ** -0.5
NCORES = 8

_cache = {}
LAST_EXEC_TIME_NS = None


def _build_nc(repeat=1):
    import concourse.bass as bass
    import concourse.mybir as mybir
    import concourse.tile as tile
    from concourse import bacc

    fp16 = mybir.dt.float16
    f32 = mybir.dt.float32
    mult = mybir.AluOpType.mult
    add = mybir.AluOpType.add

    nc = bacc.Bacc(target_bir_lowering=False, debug=False, num_devices=NCORES)

    # ---- DRAM parameters (per-core views, host-pretiled) ----
    xt_d = nc.dram_tensor("xt", [8, 128, S], fp16, kind="ExternalInput")     # x[b].T tiles
    xq_d = nc.dram_tensor("xq", [8, 128, SQ], fp16, kind="ExternalInput")    # query-half cols of x[b].T
    wq_d = nc.dram_tensor("wq", [64, 128, 128], fp16, kind="ExternalInput")  # [hp*8+Dt] tiles of qkv_w[:1024].T
    wk_d = nc.dram_tensor("wk", [64, 128, 128], fp16, kind="ExternalInput")
    wv_d = nc.dram_tensor("wv", [8, 128, FV], fp16, kind="ExternalInput")    # augmented V weights (ones slots zero)
    bq_d = nc.dram_tensor("bq", [1024], f32, kind="ExternalInput")
    bk_d = nc.dram_tensor("bk", [1024], f32, kind="ExternalInput")
    bv_d = nc.dram_tensor("bv", [FV], fp16, kind="ExternalInput")            # 1.0 at ones slots
    pw_d = nc.dram_tensor("pw", [8, 128, 1024], fp16, kind="ExternalInput")  # proj_w.T tiles
    pb_d = nc.dram_tensor("pb", [1024], f32, kind="ExternalInput")
    out_d = nc.dram_tensor("out", [SQ, D], f32, kind="ExternalOutput")

    def bcast_rows(ap, parts):
        # read the same (single-partition) data on `parts` partitions
        return bass.AP(tensor=ap.tensor, offset=ap.offset, ap=[[0, parts], *ap.ap])

    with tile.TileContext(nc) as tc:
        with (
            tc.tile_pool(name="const", bufs=1) as const,
            tc.tile_pool(name="xpool", bufs=1) as xpool,
            tc.tile_pool(name="wvpool", bufs=1) as wvpool,
            tc.tile_pool(name="acts", bufs=1) as acts,
            tc.tile_pool(name="qk", bufs=2) as qkpool,
            tc.tile_pool(name="wstream", bufs=4) as wstream,
            tc.tile_pool(name="estream", bufs=4) as estream,
            tc.tile_pool(name="small", bufs=3) as small,
            tc.tile_pool(name="outp", bufs=2) as outp,
            tc.tile_pool(name="ps", bufs=3, space="PSUM") as ps,
            tc.tile_pool(name="psO", bufs=1, space="PSUM") as psO,
        ):
            # ---- constants ----
            bvb = const.tile([128, FV], fp16, tag="bvb")
            nc.sync.dma_start(out=bvb, in_=bcast_rows(bv_d.ap(), 128))
            pbb = const.tile([128, 1024], f32, tag="pbb")
            nc.sync.dma_start(out=pbb, in_=bcast_rows(pb_d.ap(), 128))

            def body():
                xt = []
                for i in range(8):
                    t = xpool.tile([128, S], fp16, tag=f"xt{i}", name=f"xt{i}")
                    nc.sync.dma_start(out=t, in_=xt_d.ap()[i])
                    xt.append(t)
                xq = []
                for i in range(8):
                    t = xpool.tile([128, SQ], fp16, tag=f"xq{i}", name=f"xq{i}")
                    nc.sync.dma_start(out=t, in_=xq_d.ap()[i])
                    xq.append(t)
                wv = []
                for i in range(8):
                    t = wvpool.tile([128, FV], fp16, tag=f"wv{i}", name=f"wv{i}")
                    nc.sync.dma_start(out=t, in_=wv_d.ap()[i])
                    wv.append(t)

                # ---- V' = x @ wv + bv   (natural [s, f'] layout) ----
                vt = []
                for st in range(16):
                    psa = ps.tile([128, 1024], f32, tag="ps", name="psa")
                    for dt in range(8):
                        st_ap = xt[dt][:, st * 128:(st + 1) * 128]
                        nc.tensor.matmul(psa[:, 0:512], st_ap, wv[dt][:, 0:512],
                                         start=(dt == 0), stop=(dt == 7))
                        nc.tensor.matmul(psa[:, 512:1024], st_ap, wv[dt][:, 512:1024],
                                         start=(dt == 0), stop=(dt == 7))
                    psb = ps.tile([128, 1024], f32, tag="ps", name="psb")
                    for dt in range(8):
                        st_ap = xt[dt][:, st * 128:(st + 1) * 128]
                        nc.tensor.matmul(psb[:, 0:FV - 1024], st_ap, wv[dt][:, 1024:FV],
                                         start=(dt == 0), stop=(dt == 7))
                    v = acts.tile([128, FV], fp16, tag=f"v{st}", name=f"v{st}")
                    nc.vector.tensor_tensor(v[:, 0:1024], psa, bvb[:, 0:1024], add)
                    nc.vector.tensor_tensor(v[:, 1024:FV], psb[:, 0:FV - 1024],
                                            bvb[:, 1024:FV], add)
                    vt.append(v)

                otn = [acts.tile([128, SQ], fp16, tag=f"otn{i}", name=f"otn{i}")
                       for i in range(8)]

                def qk_pair(hp):
                    """QT [128f, SQ], KT [128f, S] for head pair hp."""
                    bqt = small.tile([128, 1], f32, tag="bqt", name="bqt")
                    nc.sync.dma_start(out=bqt, in_=bq_d.ap()[hp * 128:(hp + 1) * 128][:, None])
                    bkt = small.tile([128, 1], f32, tag="bkt", name="bkt")
                    nc.sync.dma_start(out=bkt, in_=bk_d.ap()[hp * 128:(hp + 1) * 128][:, None])

                    psq = ps.tile([128, 1024], f32, tag="ps", name="psq")
                    for dt in range(8):
                        w = wstream.tile([128, 128], fp16, tag="wqs", name="wq_t")
                        nc.sync.dma_start(out=w, in_=wq_d.ap()[hp * 8 + dt])
                        for c in range(2):
                            nc.tensor.matmul(psq[:, c * 512:(c + 1) * 512], w,
                                             xq[dt][:, c * 512:(c + 1) * 512],
                                             start=(dt == 0), stop=(dt == 7))
                    qt = qkpool.tile([128, SQ], fp16, tag="qt", name="qt")
                    nc.vector.tensor_scalar(qt[:], psq, bqt, None, add)

                    kt_t = qkpool.tile([128, S], fp16, tag="kt", name="kt_t")
                    for half in range(2):
                        psk = ps.tile([128, 1024], f32, tag="ps", name="psk")
                        for dt in range(8):
                            w = wstream.tile([128, 128], fp16, tag="wks", name="wk_t")
                            nc.sync.dma_start(out=w, in_=wk_d.ap()[hp * 8 + dt])
                            for c in range(2):
                                nc.tensor.matmul(
                                    psk[:, c * 512:(c + 1) * 512], w,
                                    xt[dt][:, half * 1024 + c * 512:half * 1024 + (c + 1) * 512],
                                    start=(dt == 0), stop=(dt == 7))
                        nc.vector.tensor_scalar(kt_t[:, half * 1024:(half + 1) * 1024],
                                                psk, bkt, None, add)
                    return qt, kt_t

                def attention(hp, qt, kt_t):
                    for hh in range(2):
                        h = 2 * hp + hh
                        hsl = slice(hh * 64, (hh + 1) * 64)
                        ot = psO.tile([65, SQ], f32, tag="ot", name="ot")
                        for kt in range(16):
                            sc = ps.tile([128, 1024], f32, tag="ps", name="sc")
                            for c in range(2):
                                nc.tensor.matmul(
                                    sc[:, c * 512:(c + 1) * 512],
                                    kt_t[hsl, kt * 128:(kt + 1) * 128],
                                    qt[hsl, c * 512:(c + 1) * 512],
                                    start=True, stop=True)
                            e = estream.tile([128, 1024], fp16, tag="e", name="e")
                            nc.scalar.activation(e[:], sc[:],
                                                 mybir.ActivationFunctionType.Exp,
                                                 scale=float(SCALE))
                            for c in range(2):
                                nc.tensor.matmul(
                                    ot[:, c * 512:(c + 1) * 512],
                                    vt[kt][:, h * 65:(h + 1) * 65],
                                    e[:, c * 512:(c + 1) * 512],
                                    start=(kt == 0), stop=(kt == 15))
                        rec = small.tile([1, SQ], f32, tag="rec", name="rec")
                        nc.vector.reciprocal(rec, ot[64:65, :])
                        recb = small.tile([64, SQ], f32, tag="recb", name="recb")
                        nc.gpsimd.partition_broadcast(recb, rec)
                        nc.vector.tensor_tensor(otn[hp][hh * 64:(hh + 1) * 64, :],
                                                ot[0:64, :], recb, mult)

                # one-pair-ahead pipelining: QK(i+1) emitted before attention(i)
                # so its matmuls fill PE gaps while ACT does the exps of pair i.
                pend = qk_pair(0)
                for hp in range(8):
                    nxt = qk_pair(hp + 1) if hp < 7 else None
                    attention(hp, *pend)
                    pend = nxt

                # ---- output projection ----
                pw = []
                for i in range(8):
                    t = acts.tile([128, 1024], fp16, tag=f"pw{i}", name=f"pw{i}")
                    nc.sync.dma_start(out=t, in_=pw_d.ap()[i])
                    pw.append(t)
                for st in range(8):
                    pso = ps.tile([128, 1024], f32, tag="ps", name="pso")
                    for ft in range(8):
                        for c in range(2):
                            nc.tensor.matmul(pso[:, c * 512:(c + 1) * 512],
                                             otn[ft][:, st * 128:(st + 1) * 128],
                                             pw[ft][:, c * 512:(c + 1) * 512],
                                             start=(ft == 0), stop=(ft == 7))
                    o = outp.tile([128, 1024], f32, tag="o", name="o")
                    nc.vector.tensor_tensor(o, pso, pbb, add)
                    nc.sync.dma_start(out=out_d.ap()[st * 128:(st + 1) * 128, :], in_=o)

            for _rep in range(repeat):
                body()

    nc.compile()
    return nc


def _prep_shared(qkv_w, qkv_b, proj_w, proj_b):
    f16 = np.float16
    wqT = np.ascontiguousarray(qkv_w[0:1024].T)          # [D, 1024]
    wkT = np.ascontiguousarray(qkv_w[1024:2048].T)
    # wq[hp*8+dt] = wqT[dt*128:(dt+1)*128, hp*128:(hp+1)*128]
    wq = np.ascontiguousarray(
        wqT.reshape(8, 128, 8, 128).transpose(2, 0, 1, 3).reshape(64, 128, 128)).astype(f16)
    wk = np.ascontiguousarray(
        wkT.reshape(8, 128, 8, 128).transpose(2, 0, 1, 3).reshape(64, 128, 128)).astype(f16)
    wvT = qkv_w[2048:3072].T                             # [D, 1024]
    wv = np.zeros((D, FV), np.float32)
    bv = np.zeros(FV, np.float32)
    for h in range(H):
        wv[:, h * 65:h * 65 + 64] = wvT[:, h * 64:(h + 1) * 64]
        bv[h * 65:h * 65 + 64] = qkv_b[2048 + h * 64:2048 + (h + 1) * 64]
        bv[h * 65 + 64] = 1.0
    wv = np.ascontiguousarray(wv.reshape(8, 128, FV)).astype(f16)
    pw = np.ascontiguousarray(proj_w.T.reshape(8, 128, 1024)).astype(f16)
    return dict(
        wq=wq, wk=wk, wv=wv,
        bq=np.ascontiguousarray(qkv_b[0:1024]).astype(np.float32),
        bk=np.ascontiguousarray(qkv_b[1024:2048]).astype(np.float32),
        bv=bv.astype(f16),
        pw=pw,
        pb=np.ascontiguousarray(proj_b).astype(np.float32),
    )


def _make_in_maps(x, qkv_w, qkv_b, proj_w, proj_b):
    x = np.asarray(x, np.float32)
    shared = _prep_shared(np.asarray(qkv_w, np.float32), np.asarray(qkv_b, np.float32),
                          np.asarray(proj_w, np.float32), np.asarray(proj_b, np.float32))
    in_maps = []
    for c in range(NCORES):
        b, half = c // 2, c % 2
        xT = np.ascontiguousarray(x[b].T).astype(np.float16)          # [D, S]
        xq = np.ascontiguousarray(xT[:, half * SQ:(half + 1) * SQ])   # [D, SQ]
        m = dict(shared)
        m["xt"] = np.ascontiguousarray(xT.reshape(8, 128, S))
        m["xq"] = np.ascontiguousarray(xq.reshape(8, 128, SQ))
        in_maps.append(m)
    return in_maps


def kernel(x, qkv_w, qkv_b, proj_w, proj_b):
    global LAST_EXEC_TIME_NS
    from concourse.bass_utils import run_bass_kernel_spmd

    in_maps = _make_in_maps(x, qkv_w, qkv_b, proj_w, proj_b)
    if "nc" not in _cache:
        _cache["nc"] = _build_nc()
    nc = _cache["nc"]

    res = run_bass_kernel_spmd(nc, in_maps, core_ids=list(range(NCORES)))
    LAST_EXEC_TIME_NS = res.exec_time_ns

    out = np.zeros((B, S, D), np.float32)
    for c in range(NCORES):
        b, half = c // 2, c % 2
        out[b, half * SQ:(half + 1) * SQ, :] = res.results[c]["out"]
    return out


# revision 9
# speedup vs baseline: 1.0407x; 1.0407x over previous
"""Multi-head self-attention on 8 TRN2 NeuronCores — v3.

Same math/sharding as v1/v2 ((batch x query-half) shards, fp16 compute,
transposed-scores softmax with ones-column denominators), but restructured
for this platform's cost profile (measured: ~50us per matmul instruction,
~10us per DVE op, ACT ~free): matmul count minimized via N=1024 moving
operands, all inputs shipped in exact SBUF layout and loaded with one
contiguous DMA each.
"""

import os
import numpy as np

B, S, D = 4, 2048, 1024
H, DK = 16, 64
SQ = S // 2
FV = H * 65          # V' columns incl. per-head ones column
SCALE = 64 ** -0.5
NCORES = 8

_cache = {}
LAST_EXEC_TIME_NS = None

MMN = int(os.environ.get("KERNEL_MMN", "512"))   # moving free dim per matmul


def _build_nc(repeat=1):
    import concourse.bass as bass
    import concourse.mybir as mybir
    import concourse.tile as tile
    from concourse import bacc

    fp16 = mybir.dt.float16
    f32 = mybir.dt.float32
    mult = mybir.AluOpType.mult
    add = mybir.AluOpType.add

    nc = bacc.Bacc(target_bir_lowering=False, debug=False, num_devices=NCORES)

    # ---- DRAM parameters, already in SBUF layout ----
    xt_d = nc.dram_tensor("xt", [128, 8, S], fp16, kind="ExternalInput")
    xq_d = nc.dram_tensor("xq", [128, 8, SQ], fp16, kind="ExternalInput")
    wq_d = nc.dram_tensor("wq", [128, 64, 128], fp16, kind="ExternalInput")
    wk_d = nc.dram_tensor("wk", [128, 64, 128], fp16, kind="ExternalInput")
    wv_d = nc.dram_tensor("wv", [128, 8, 1024], fp16, kind="ExternalInput")  # dense V wT
    bq_d = nc.dram_tensor("bq", [128, 8], f32, kind="ExternalInput")
    bk_d = nc.dram_tensor("bk", [128, 8], f32, kind="ExternalInput")
    bv_d = nc.dram_tensor("bv", [1024], fp16, kind="ExternalInput")          # dense V bias
    pw_d = nc.dram_tensor("pw", [128, 8, 1024], fp16, kind="ExternalInput")
    pb_d = nc.dram_tensor("pb", [1024], f32, kind="ExternalInput")
    out_d = nc.dram_tensor("out", [SQ, D], f32, kind="ExternalOutput")

    def bcast_rows(ap, parts):
        return bass.AP(tensor=ap.tensor, offset=ap.offset, ap=[[0, parts], *ap.ap])

    def mm_chunks(total):
        c = []
        o = 0
        while o < total:
            n = min(MMN, total - o)
            c.append((o, n))
            o += n
        return c

    with tile.TileContext(nc) as tc:
        with (
            tc.tile_pool(name="const", bufs=1) as const,
            tc.tile_pool(name="xpool", bufs=1) as xpool,
            tc.tile_pool(name="acts", bufs=1) as acts,
            tc.tile_pool(name="qk", bufs=2) as qkpool,
            tc.tile_pool(name="estream", bufs=4) as estream,
            tc.tile_pool(name="small", bufs=3) as small,
            tc.tile_pool(name="ps", bufs=2, space="PSUM") as ps,
            tc.tile_pool(name="psO", bufs=2, space="PSUM") as psO,
            tc.tile_pool(name="dscr", bufs=2, space="DRAM") as dscr,
        ):
            bvb = const.tile([128, 1024], fp16, tag="bvb")
            nc.sync.dma_start(out=bvb, in_=bcast_rows(bv_d.ap(), 128))
            pbb = const.tile([128, 1024], f32, tag="pbb")
            nc.sync.dma_start(out=pbb, in_=bcast_rows(pb_d.ap(), 128))
            wq_all = const.tile([128, 64, 128], fp16, tag="wq_all")
            nc.sync.dma_start(out=wq_all, in_=wq_d.ap())
            wk_all = const.tile([128, 64, 128], fp16, tag="wk_all")
            nc.sync.dma_start(out=wk_all, in_=wk_d.ap())
            bq_all = const.tile([128, 8], f32, tag="bq_all")
            nc.sync.dma_start(out=bq_all, in_=bq_d.ap())
            bk_all = const.tile([128, 8], f32, tag="bk_all")
            nc.sync.dma_start(out=bk_all, in_=bk_d.ap())

            def body():
                xt = xpool.tile([128, 8, S], fp16, tag="xt", name="xt")
                nc.sync.dma_start(out=xt, in_=xt_d.ap())
                xq = xpool.tile([128, 8, SQ], fp16, tag="xq", name="xq")
                nc.sync.dma_start(out=xq, in_=xq_d.ap())
                # pw later reuses wv's slot (same tag) once V' is done
                wv = xpool.tile([128, 8, 1024], fp16, tag="wv", name="wv", bufs=1)
                nc.sync.dma_start(out=wv, in_=wv_d.ap())

                # ---- V' ----
                vt = []
                for st in range(16):
                    psa = ps.tile([128, 1024], f32, tag="ps", name="psa")
                    for dt in range(8):
                        for o, n in mm_chunks(1024):
                            nc.tensor.matmul(psa[:, o:o + n],
                                             xt[:, dt, st * 128:(st + 1) * 128],
                                             wv[:, dt, o:o + n],
                                             start=(dt == 0), stop=(dt == 7))
                    v = acts.tile([128, 16, 65], fp16, tag=f"v{st}", name=f"v{st}")
                    # dense [128,1024] psum + bias -> strided 64-col blocks of v
                    nc.vector.tensor_tensor(
                        v[:, :, 0:64],
                        psa.rearrange("p (a b) -> p a b", a=16),
                        bvb.rearrange("p (a b) -> p a b", a=16), add)
                    nc.vector.memset(v[:, :, 64], 1.0)
                    vt.append(v)

                otn = [acts.tile([128, SQ], fp16, tag=f"otn{i}", name=f"otn{i}")
                       for i in range(8)]

                def qk_pair(hp):
                    psq = ps.tile([128, SQ], f32, tag="ps", name="psq")
                    for dt in range(8):
                        for o, n in mm_chunks(SQ):
                            nc.tensor.matmul(psq[:, o:o + n],
                                             wq_all[:, hp * 8 + dt, :],
                                             xq[:, dt, o:o + n],
                                             start=(dt == 0), stop=(dt == 7))
                    qt = qkpool.tile([128, SQ], fp16, tag="qt", name="qt")
                    nc.vector.tensor_scalar(qt[:], psq, bq_all[:, hp:hp + 1], None, add)

                    kt_t = qkpool.tile([128, S], fp16, tag="kt", name="kt_t")
                    for half in range(2):
                        psk = ps.tile([128, SQ], f32, tag="ps", name="psk")
                        for dt in range(8):
                            for o, n in mm_chunks(SQ):
                                nc.tensor.matmul(psk[:, o:o + n],
                                                 wk_all[:, hp * 8 + dt, :],
                                                 xt[:, dt, half * SQ + o:half * SQ + o + n],
                                                 start=(dt == 0), stop=(dt == 7))
                        nc.vector.tensor_scalar(kt_t[:, half * SQ:(half + 1) * SQ],
                                                psk, bk_all[:, hp:hp + 1], None, add)
                    return qt, kt_t

                def attention(hp, qt, kt_t):
                    ot2 = []
                    for hh in range(2):
                        ot = psO.tile([65, SQ], f32, tag="ot", name=f"ot{hh}")
                        ot2.append(ot)
                    for kt in range(16):
                        sc2 = []
                        for hh in range(2):
                            sc = ps.tile([128, SQ], f32, tag="ps", name=f"sc{hh}")
                            sc2.append(sc)
                        for o, n in mm_chunks(SQ):
                            for hh in range(2):
                                hsl = slice(hh * 64, (hh + 1) * 64)
                                nc.tensor.matmul(
                                    sc2[hh][:, o:o + n],
                                    kt_t[hsl, kt * 128:(kt + 1) * 128],
                                    qt[hsl, o:o + n],
                                    start=True, stop=True)
                        for hh in range(2):
                            h = 2 * hp + hh
                            e = estream.tile([128, SQ], fp16, tag="e", name="e")
                            nc.scalar.activation(e[:], sc2[hh][:],
                                                 mybir.ActivationFunctionType.Exp,
                                                 scale=float(SCALE))
                            for o, n in mm_chunks(SQ):
                                nc.tensor.matmul(
                                    ot2[hh][:, o:o + n],
                                    vt[kt][:, h, :],
                                    e[:, o:o + n],
                                    start=(kt == 0), stop=(kt == 15))
                    for hh in range(2):
                        ot = ot2[hh]
                        rec = small.tile([1, SQ], f32, tag="rec", name="rec")
                        nc.vector.reciprocal(rec, ot[64:65, :])
                        recb = small.tile([64, SQ], f32, tag="recb", name="recb")
                        nc.gpsimd.partition_broadcast(recb, rec)
                        nc.vector.tensor_tensor(otn[hp][hh * 64:(hh + 1) * 64, :],
                                                ot[0:64, :], recb, mult)

                pend = qk_pair(0)
                for hp in range(8):
                    nxt = qk_pair(hp + 1) if hp < 7 else None
                    attention(hp, *pend)
                    pend = nxt

                # ---- output projection ----
                pw = xpool.tile([128, 8, 1024], fp16, tag="wv", name="pw", bufs=1)
                nc.sync.dma_start(out=pw, in_=pw_d.ap())
                for st in range(8):
                    pso = ps.tile([128, 1024], f32, tag="ps", name="pso")
                    for ft in range(8):
                        for o, n in mm_chunks(1024):
                            nc.tensor.matmul(pso[:, o:o + n],
                                             otn[ft][:, st * 128:(st + 1) * 128],
                                             pw[:, ft, o:o + n],
                                             start=(ft == 0), stop=(ft == 7))
                    o_t = small.tile([128, 1024], f32, tag="o_t", name="o_t", bufs=2)
                    nc.vector.tensor_tensor(o_t, pso, pbb, add)
                    nc.sync.dma_start(out=out_d.ap()[st * 128:(st + 1) * 128, :], in_=o_t)

            for _rep in range(repeat):
                body()

    nc.compile()
    return nc


def _prep_shared(qkv_w, qkv_b, proj_w, proj_b):
    f16 = np.float16
    wqT = np.ascontiguousarray(qkv_w[0:1024].T)          # [D, 1024]
    wkT = np.ascontiguousarray(qkv_w[1024:2048].T)
    wvT = np.ascontiguousarray(qkv_w[2048:3072].T)
    # wq_all[p, hp*8+dt, c] = wqT[dt*128+p, hp*128+c]
    wq = np.ascontiguousarray(
        wqT.reshape(8, 128, 8, 128).transpose(1, 2, 0, 3).reshape(128, 64, 128)).astype(f16)
    wk = np.ascontiguousarray(
        wkT.reshape(8, 128, 8, 128).transpose(1, 2, 0, 3).reshape(128, 64, 128)).astype(f16)
    # wv[p, dt, f] = wvT[dt*128+p, f] ; V' ones handled on-device by memset
    wv = np.ascontiguousarray(
        wvT.reshape(8, 128, 1024).transpose(1, 0, 2)).astype(f16)
    pw = np.ascontiguousarray(
        proj_w.T.reshape(8, 128, 1024).transpose(1, 0, 2)).astype(f16)
    bq = np.ascontiguousarray(qkv_b[0:1024].reshape(8, 128).T).astype(np.float32)
    bk = np.ascontiguousarray(qkv_b[1024:2048].reshape(8, 128).T).astype(np.float32)
    return dict(
        wq=wq, wk=wk, wv=wv, bq=bq, bk=bk,
        bv=np.ascontiguousarray(qkv_b[2048:3072]).astype(f16),
        pw=pw,
        pb=np.ascontiguousarray(proj_b).astype(np.float32),
    )


def _make_in_maps(x, qkv_w, qkv_b, proj_w, proj_b):
    x = np.asarray(x, np.float32)
    shared = _prep_shared(np.asarray(qkv_w, np.float32), np.asarray(qkv_b, np.float32),
                          np.asarray(proj_w, np.float32), np.asarray(proj_b, np.float32))
    in_maps = []
    for c in range(NCORES):
        b, half = c // 2, c % 2
        xT = np.ascontiguousarray(x[b].T).astype(np.float16)          # [D, S]
        m = dict(shared)
        m["xt"] = np.ascontiguousarray(xT.reshape(8, 128, S).transpose(1, 0, 2))
        m["xq"] = np.ascontiguousarray(
            xT[:, half * SQ:(half + 1) * SQ].reshape(8, 128, SQ).transpose(1, 0, 2))
        in_maps.append(m)
    return in_maps


def kernel(x, qkv_w, qkv_b, proj_w, proj_b):
    global LAST_EXEC_TIME_NS
    from concourse.bass_utils import run_bass_kernel_spmd

    in_maps = _make_in_maps(x, qkv_w, qkv_b, proj_w, proj_b)
    if "nc" not in _cache:
        _cache["nc"] = _build_nc()
    nc = _cache["nc"]

    res = run_bass_kernel_spmd(nc, in_maps, core_ids=list(range(NCORES)))
    LAST_EXEC_TIME_NS = res.exec_time_ns

    out = np.zeros((B, S, D), np.float32)
    for c in range(NCORES):
        b, half = c // 2, c % 2
        out[b, half * SQ:(half + 1) * SQ, :] = res.results[c]["out"]
    return out
